# revision 19
# baseline (speedup 1.0000x reference)
"""Trainium2 Bass kernel for nn_Cross_AttentionHead_withMask.

Cross-attention head: q = rope(x_text @ Wq.T), k = rope2d(x_image @ Wk.T),
v = x_image @ Wv.T, out = softmax(q k^T / sqrt(512)) v.
(x_latex_mask is accepted but unused — it is dead in the reference.)

Sharding: data-parallel over batch B=8, one batch per NeuronCore (8 cores).

Per-core device program (all matmuls bf16, accumulation/softmax stats fp32):
  - host ships x_image[b].T / x_text[b].T (bf16) so the contraction dim (C)
    lands on SBUF partitions without any on-device transposes
  - head dim is permuted to evens-then-odds so RoPE pairs become the row
    blocks [0:32] / [32:64]; rope = A*CC + partner(A)*SS (2 muls + 1 add)
  - scores computed transposed: weiT[t, s] = K2[:, t-tile].T @ Q2[:, s-chunk]
  - softmax exp is SPLIT between the Activation engine (exact exp) and the
    DVE (Schraudolph fast-exp: y = round(x*log2e*128 + (127*128 - C)) as
    int16, bit-viewed as bf16), alternating per step so neither engine
    paces the PE
  - attention-out: outT[h, s] += v_aug[t-tile].T @ expT, where v_aug carries
    a ones column so row 64 accumulates the softmax denominator for free;
    att-out lags the scores by TWO steps so exp latency is fully hidden
  - PSUM->SBUF projection copies and rope partner-swaps run on GpSimd (Pool),
    keeping Scalar exp-only and DVE mostly exp+rope-muls
  - epilogue: PE-transpose [65, 128] -> [128, 65], per-partition reciprocal
    of the Z column, tensor_scalar multiply, DMA out
"""
import numpy as np
from contextlib import ExitStack

import ml_dtypes

B, TQ, TK = 8, 2048, 4096
DIM_IMG, DIM_TXT, HS = 512, 128, 64
N_CORES = 8
SCALE = float(DIM_IMG) ** -0.5  # reference scales by sqrt(image embed dim)

# DVE fast-exp constants: exp(SCALE*x) ~= bitview_bf16(int16(round(
#   x * SCALE*log2(e)*128 + (127*128 - FEXP_C))))
FEXP_C = 6.0
FEXP_SCALE = SCALE * float(np.log2(np.e)) * 128.0
FEXP_BIAS = 127.0 * 128.0 - FEXP_C

BF16 = ml_dtypes.bfloat16

_prog_cache = {}


def _patch_tile_drain():
    """This walrus build rejects a Drain carrying >1 sem wait; split the
    TileContext exit waits onto one-wait NoOps."""
    import concourse.tile as tile
    from concourse import mybir
    from concourse.vector_clock import ScopedClock

    if getattr(tile.TileContext, "_drain_patched", False):
        return

    def _drain_and_barrier(self, tick_clock, wait_clock):
        nc = self.nc
        nop = nc.sync.nop()
        wait_clock.add_sem_waits(nop.ins, ScopedClock({None: tick_clock.global_clock}))
        si = nop.ins.sync_info
        waits = list(si.on_wait) if si is not None else []
        if len(waits) > 1:
            nop.ins.sync_info = mybir.SyncInfo(on_wait=[waits[0]], on_update=[])
            for w in waits[1:]:
                extra = nc.sync.nop()
                extra.ins.sync_info = mybir.SyncInfo(on_wait=[w], on_update=[])
        nc.sync.drain()
        nc.all_engine_barrier()
        assert self.sems is not None
        popped = nc._tile_sem_poison_stack.pop()
        assert popped is self._sem_poison
        nc.clear_and_free_semaphores(list(self.sems.allocated().values()))
        nc.all_engine_barrier()

    tile.TileContext._drain_and_barrier = _drain_and_barrier
    tile.TileContext._drain_patched = True


def _split_excess_waits(nc):
    """This walrus build caps sem waits per instruction (1 for DMA/Drain-style
    control instructions, 2 for compute). Move excess waits onto same-engine
    NoOps inserted right before the offending instruction — the engine queue
    is FIFO, so blocking dispatch on the NoOp is semantically equivalent."""
    from concourse import mybir

    ctr = 0
    for fn in nc.m.functions:
        for b in fn.blocks:
            il = b.instructions
            new = []
            changed = False
            for inst in il:
                si = inst.sync_info
                waits = list(si.on_wait) if si is not None else []
                lim = 1
                if len(waits) > lim:
                    for w in waits[lim:]:
                        nop = mybir.InstNoOp(name=f"wsplit-{ctr}", ins=[], outs=[])
                        ctr += 1
                        nop.engine = inst.engine
                        nop.sync_info = mybir.SyncInfo(on_wait=[w], on_update=[])
                        new.append(nop)
                    inst.sync_info = mybir.SyncInfo(
                        on_wait=waits[:lim], on_update=list(si.on_update)
                    )
                    changed = True
                new.append(inst)
            if changed:
                b.instructions = new


def build_program(split_waits=True):
    """Build the single-core Bass program (same program runs SPMD on 8 cores)."""
    key = ("nc", split_waits)
    if key in _prog_cache:
        return _prog_cache[key]

    _patch_tile_drain()
    import concourse.bass as bass
    import concourse.tile as tile
    from concourse import mybir
    from concourse.masks import make_identity

    FP = mybir.dt.float32
    BF = mybir.dt.bfloat16
    I16 = mybir.dt.int16
    MULT = mybir.AluOpType.mult
    ADD = mybir.AluOpType.add

    nc = bass.Bass("TRN2", target_bir_lowering=False, debug=False)
    xt = nc.dram_tensor("xt", [DIM_IMG, TK], BF, kind="ExternalInput").ap()
    xtt = nc.dram_tensor("xtt", [DIM_TXT, TQ], BF, kind="ExternalInput").ap()
    wk = nc.dram_tensor("wk", [DIM_IMG, HS], BF, kind="ExternalInput").ap()
    wq = nc.dram_tensor("wq", [DIM_TXT, HS], BF, kind="ExternalInput").ap()
    wv = nc.dram_tensor("wv", [DIM_IMG, HS], BF, kind="ExternalInput").ap()
    cck = nc.dram_tensor("cck", [HS, TK], BF, kind="ExternalInput").ap()
    ssk = nc.dram_tensor("ssk", [HS, TK], BF, kind="ExternalInput").ap()
    ccq = nc.dram_tensor("ccq", [HS, TQ], BF, kind="ExternalInput").ap()
    ssq = nc.dram_tensor("ssq", [HS, TQ], BF, kind="ExternalInput").ap()
    out = nc.dram_tensor("out", [TQ, HS], FP, kind="ExternalOutput").ap()

    Exp = mybir.ActivationFunctionType.Exp
    NC4 = DIM_IMG // 128  # 4 c-chunks
    NT = TK // 128  # 32 t-tiles
    NSC = TQ // 512  # 4 s-chunks

    with tile.TileContext(nc) as tc:
        with ExitStack() as ctx:
            const = ctx.enter_context(tc.tile_pool(name="const", bufs=1))
            pwp = ctx.enter_context(tc.tile_pool(name="pw", bufs=3, space="PSUM"))
            pop = ctx.enter_context(tc.tile_pool(name="po", bufs=2, space="PSUM"))
            esb = ctx.enter_context(tc.tile_pool(name="esb", bufs=5))
            osbp = ctx.enter_context(tc.tile_pool(name="osb", bufs=2))

            # ---- DMA rings: the 4 MB x_image.T alone on the fast HWDGE (sync)
            # ring; everything small on the gpsimd SWDGE ring ----
            xtt_sb = const.tile([128, TQ], BF, tag="xtt")
            nc.sync.dma_start(xtt_sb[:], xtt[:])
            xt_sb = [const.tile([128, TK], BF, tag=f"xt{ci}", name=f"xt_sb{ci}")
                     for ci in range(NC4)]
            # column-chunk order so k/v-proj can start as soon as the first
            # 512 columns of all four c-blocks have landed
            for j in range(8):
                cs = slice(j * 512, (j + 1) * 512)
                for ci in range(NC4):
                    nc.sync.dma_start(xt_sb[ci][:, cs], xt[ci * 128 : (ci + 1) * 128, cs])
            wq_sb = const.tile([128, HS], BF, tag="wq")
            nc.gpsimd.dma_start(wq_sb[:], wq[:])
            wk_sb = const.tile([128, NC4 * HS], BF, tag="wk")
            nc.gpsimd.dma_start(
                wk_sb[:].rearrange("p (a h) -> p a h", a=NC4),
                wk.rearrange("(a p) h -> p a h", p=128),
            )
            wv_sb = const.tile([128, NC4 * HS], BF, tag="wv")
            nc.gpsimd.dma_start(
                wv_sb[:].rearrange("p (a h) -> p a h", a=NC4),
                wv.rearrange("(a p) h -> p a h", p=128),
            )
            ccq_sb = const.tile([HS, TQ], BF, tag="ccq")
            nc.gpsimd.dma_start(ccq_sb[:], ccq[:])
            ssq_sb = const.tile([HS, TQ], BF, tag="ssq")
            nc.gpsimd.dma_start(ssq_sb[:], ssq[:])
            cck_sb = const.tile([HS, TK], BF, tag="cck")
            ssk_sb = const.tile([HS, TK], BF, tag="ssk")
            nc.gpsimd.dma_start(cck_sb[:], cck[:])
            nc.gpsimd.dma_start(ssk_sb[:], ssk[:])
            ident = const.tile([128, 128], FP, tag="ident")

            kt_pre = const.tile([HS, TK], BF, tag="ktpre")
            qt_pre = const.tile([HS, TQ], BF, tag="qtpre")
            v_half = [const.tile([128, NT * 65 // 2], BF, tag=f"vall{h}", name=f"vall{h}")
                      for h in range(2)]
            K2h = [const.tile([128, TK // 2], BF, tag=f"K2{h}", name=f"K2{h}")
                   for h in range(2)]
            pk = const.tile([HS, TK], BF, tag="pk")
            pq = const.tile([HS, TQ], BF, tag="pq")
            t1k = const.tile([HS, TK], BF, tag="t1k")
            t2k = const.tile([HS, TK], BF, tag="t2k")

            # ---- q projection + rope ----
            for j in range(TQ // 512):
                ps = pwp.tile([HS, 512], FP, tag="psw", name=f"psq{j}")
                nc.tensor.matmul(
                    ps[:], lhsT=wq_sb[:], rhs=xtt_sb[:, j * 512 : (j + 1) * 512],
                    start=True, stop=True,
                )
                nc.scalar.copy(qt_pre[:, j * 512 : (j + 1) * 512], ps[:])
            # rope partner-swaps and row-duplications are pure SBUF shuffles:
            # run them on the gpsimd DMA ring (free, and not stuck behind the
            # 4 MB x_image stream on the sync ring)
            nc.gpsimd.dma_start(pq[0:32, :], qt_pre[32:64, :])
            nc.gpsimd.dma_start(pq[32:64, :], qt_pre[0:32, :])
            t1q = const.tile([HS, TQ], BF, tag="t1q")
            nc.vector.tensor_mul(t1q[:], qt_pre[:], ccq_sb[:])
            t2q = const.tile([HS, TQ], BF, tag="t2q")
            nc.vector.tensor_mul(t2q[:], pq[:], ssq_sb[:])
            Q2 = const.tile([128, TQ], BF, tag="Q2")
            nc.vector.tensor_add(Q2[0:HS, :], t1q[:], t2q[:])
            nc.gpsimd.dma_start(Q2[HS:128, :], Q2[0:HS, :])

            def k_proj_chunk(j, cp):
                ps = pwp.tile([HS, 512], FP, tag="psw", name=f"psk{j}")
                for ci in range(NC4):
                    nc.tensor.matmul(
                        ps[:],
                        lhsT=wk_sb[:, ci * HS : (ci + 1) * HS],
                        rhs=xt_sb[ci][:, j * 512 : (j + 1) * 512],
                        start=(ci == 0), stop=(ci == NC4 - 1),
                    )
                cp(kt_pre[:, j * 512 : (j + 1) * 512], ps[:])

            def k_rope_half(h):
                cs = slice(h * (TK // 2), (h + 1) * (TK // 2))
                nc.gpsimd.dma_start(pk[0:32, cs], kt_pre[32:64, cs])
                nc.gpsimd.dma_start(pk[32:64, cs], kt_pre[0:32, cs])
                nc.vector.tensor_mul(t1k[:, cs], kt_pre[:, cs], cck_sb[:, cs])
                nc.vector.tensor_mul(t2k[:, cs], pk[:, cs], ssk_sb[:, cs])
                nc.vector.tensor_add(K2h[h][0:HS, :], t1k[:, cs], t2k[:, cs])
                nc.gpsimd.dma_start(K2h[h][HS:128, :], K2h[h][0:HS, :])

            def v_proj_tile(tt, cp):
                ps = pwp.tile([128, HS], FP, tag="psw", name=f"psv{tt}")
                for ci in range(NC4):
                    nc.tensor.matmul(
                        ps[:],
                        lhsT=xt_sb[ci][:, tt * 128 : (tt + 1) * 128],
                        rhs=wv_sb[:, ci * HS : (ci + 1) * HS],
                        start=(ci == 0), stop=(ci == NC4 - 1),
                    )
                vh, vo = v_half[tt // (NT // 2)], (tt % (NT // 2)) * 65
                cp(vh[:, vo : vo + HS], ps[:])

            # ---- attention machinery (flat pipeline over (sc, group) steps) ----
            GROUPS = [2] * 16
            psos = {}
            # att-out lags scores/exp by TWO steps so exp latency never
            # stalls the PE; pend queue holds up to 2 outstanding groups
            state = {"pend": [], "pend_epi": None}

            def att_group(pend):
                psc, pet, pgn, ptt = pend
                for j in range(pgn):
                    tj = ptt + j
                    vh, vo = v_half[tj // (NT // 2)], (tj % (NT // 2)) * 65
                    nc.tensor.matmul(
                        psos[psc][:],
                        lhsT=vh[:, vo : vo + 65],
                        rhs=pet[:, j * 512 : (j + 1) * 512],
                        start=(tj == 0), stop=(tj == NT - 1),
                    )

            def epilogue(psc):
                pso = psos.pop(psc)
                osb = osbp.tile([65, 512], FP, tag="osb", name=f"osb{psc}")
                nc.scalar.copy(osb[:], pso[:])
                out_sb = osbp.tile([128, 4 * HS], FP, tag="outsb", name=f"outsb{psc}")
                for j in range(4):
                    pst = pwp.tile([128, 65], FP, tag="psw", name=f"pst{psc}_{j}")
                    nc.tensor.transpose(
                        pst[:], osb[:, j * 128 : (j + 1) * 128], ident[0:65, 0:65]
                    )
                    zr = osbp.tile([128, 1], FP, tag="zr", name=f"zr{psc}_{j}")
                    nc.vector.reciprocal(zr[:], pst[:, HS : HS + 1])
                    nc.vector.tensor_scalar_mul(
                        out_sb[:, j * HS : (j + 1) * HS], pst[:, 0:HS], zr[:]
                    )
                nc.sync.dma_start(
                    out[psc * 512 : (psc + 1) * 512, :].rearrange(
                        "(j p) h -> p j h", p=128
                    ),
                    out_sb[:].rearrange("p (j h) -> p j h", j=4),
                )

            def flush_one():
                if not state["pend"]:
                    return
                pend = state["pend"].pop(0)
                psc, _, pgn, ptt = pend
                if psc not in psos:
                    psos[psc] = pop.tile([65, 512], FP, tag="pso", name=f"pso{psc}")
                att_group(pend)
                if state["pend_epi"] is not None:
                    epilogue(state["pend_epi"])
                    state["pend_epi"] = None
                if ptt + pgn == NT:
                    state["pend_epi"] = psc

            def att_steps(steps, exp_eng, extra=None):
                for si, (sc, gi) in enumerate(steps):
                    gn = GROUPS[gi]
                    tt = sum(GROUPS[:gi])
                    psw = pwp.tile([128, 1024], FP, tag="psw", name=f"psw{sc}_{gi}")
                    et = esb.tile([128, 1024], BF, tag="et", name=f"et{sc}_{gi}")
                    if si % 2 == 0:
                        # light HAM keep-warm filler on the psw slot-wait
                        nc.tensor.matmul(
                            psw[0:HS, 0:256], lhsT=wq_sb[:], rhs=xtt_sb[:, 0:256],
                            start=True, stop=True,
                        )
                    for j in range(gn):
                        tj = tt + j
                        kh = K2h[tj // (NT // 2)]
                        ko = (tj % (NT // 2)) * 128
                        rb = (j % 2) * HS  # alternate PE row groups: hides ldweights
                        nc.tensor.matmul(
                            psw[:, j * 512 : (j + 1) * 512],
                            lhsT=kh[rb : rb + HS, ko : ko + 128],
                            rhs=Q2[rb : rb + HS, sc * 512 : (sc + 1) * 512],
                            start=True, stop=True,
                        )
                    if exp_eng(si) == "scalar":
                        nc.scalar.activation(
                            et[:, : gn * 512], psw[:, : gn * 512], Exp, scale=SCALE
                        )
                    else:
                        # DVE fast-exp: int16(round(x*a + b)) bit-viewed as bf16
                        nc.vector.tensor_scalar(
                            et[:, : gn * 512].bitcast(I16),
                            psw[:, : gn * 512],
                            FEXP_SCALE, FEXP_BIAS, MULT, ADD,
                        )
                    if extra is not None:
                        extra(si)
                    if len(state["pend"]) >= 2:
                        flush_one()
                    state["pend"].append((sc, et, gn, tt))

            # ---- interleaved emission: first halves of k/v + rope, then the
            # first half of sc0's attention (exp engines start early),
            # then the second halves, then the rest of the attention ----
            # PE warm-up: dependency-free fillers right after q-proj so the
            # clock gate is already at 8/8 when x_image lands and k/v-proj run
            # Pre-phase: consume x_image chunk-by-chunk as the DMA delivers it
            # (~2.7us per 512-col chunk at the ~190GB/s HBM share). Interleave
            # dependency-free filler matmuls so the PE never looks idle to the
            # HAM clock governor during the DMA window — a throttled window
            # would also halve the DMA rate (death spiral).
            garb0 = pwp.tile([HS, 512], FP, tag="psw", name="garb0")

            def garb(n):
                for fi in range(n):
                    nc.tensor.matmul(
                        garb0[:], lhsT=wq_sb[:], rhs=xtt_sb[:, 0:512],
                        start=True, stop=True,
                    )

            garb(10)
            for j in range(4):
                k_proj_chunk(j, nc.scalar.copy)
                for tt in range(4 * j, 4 * j + 4):
                    v_proj_tile(tt, nc.vector.tensor_copy)
                garb(4)
            k_rope_half(0)
            # identity + v_aug ones columns: needed from the first epilogue /
            # att-out; emitted here so they don't block the gpsimd DMA ring
            make_identity(nc, ident[:])
            nc.gpsimd.memset(v_half[0][:, HS :: 65], 1.0)
            nc.gpsimd.memset(v_half[1][:, HS :: 65], 1.0)

            # exp-engine schedule: Scalar-only while DVE ropes (early sc0),
            # then alternate DVE/Scalar
            def exp_eng_sc0(si):
                if si < 8:
                    return "scalar"
                return "vector" if si % 2 == 0 else "scalar"

            def exp_eng_rest(si):
                return "vector" if si % 2 == 0 else "scalar"

            def h1_proj_extra(si):
                # spread the k-half1 / v-half1 projections through sc0's
                # attention steps, aligned with x_image chunk arrival;
                # PSUM->SBUF copies alternate Scalar/DVE
                cp = (nc.vector.tensor_copy if si % 2 == 0 else nc.scalar.copy)
                if si < 4:
                    k_proj_chunk(4 + si, cp)
                    v_proj_tile(16 + 2 * si, cp)
                    v_proj_tile(17 + 2 * si, cp)
                elif si == 4:
                    k_rope_half(1)
                elif si < 9:
                    t0_ = 24 + (si - 5) * 2
                    v_proj_tile(t0_, cp)
                    v_proj_tile(t0_ + 1, cp)

            att_steps([(0, gi) for gi in range(16)], exp_eng_sc0,
                      extra=h1_proj_extra)
            att_steps([(sc, gi) for sc in range(1, NSC) for gi in range(16)],
                      exp_eng_rest)
            # flush remaining groups + epilogues
            while state["pend"]:
                flush_one()
            if state["pend_epi"] is not None:
                epilogue(state["pend_epi"])

    if split_waits:
        _split_excess_waits(nc)
    _prog_cache[key] = nc
    return nc


def make_in_maps(x_image, x_text_emb, freqs_latex, freqs_img_x, freqs_img_y, Wk, Wq, Wv):
    """Host-side prep: transpose/cast activations, permute+transpose weights,
    build rope cos/sin tables in the permuted row layout."""
    perm = np.concatenate([np.arange(0, HS, 2), np.arange(1, HS, 2)])

    wk_dev = np.ascontiguousarray(np.asarray(Wk)[perm].T).astype(BF16)
    wq_dev = np.ascontiguousarray(np.asarray(Wq)[perm].T).astype(BF16)
    wv_dev = np.ascontiguousarray(np.asarray(Wv).T).astype(BF16)

    fx = np.asarray(freqs_img_x, dtype=np.float32)
    fy = np.asarray(freqs_img_y, dtype=np.float32)
    fl = np.asarray(freqs_latex, dtype=np.float32)
    ck_half = np.concatenate([fx[:, :, 0].T, fy[:, :, 0].T], axis=0)  # [32, TK]
    sk_half = np.concatenate([fx[:, :, 1].T, fy[:, :, 1].T], axis=0)
    cck = np.ascontiguousarray(np.concatenate([ck_half, ck_half], 0)).astype(BF16)
    ssk = np.ascontiguousarray(np.concatenate([-sk_half, sk_half], 0)).astype(BF16)
    cq_half = fl[:, :, 0].T  # [32, TQ]
    sq_half = fl[:, :, 1].T
    ccq = np.ascontiguousarray(np.concatenate([cq_half, cq_half], 0)).astype(BF16)
    ssq = np.ascontiguousarray(np.concatenate([-sq_half, sq_half], 0)).astype(BF16)

    xi = np.asarray(x_image, dtype=np.float32)
    xte = np.asarray(x_text_emb, dtype=np.float32)
    in_maps = []
    for b in range(N_CORES):
        in_maps.append(
            {
                "xt": np.ascontiguousarray(xi[b].T).astype(BF16),
                "xtt": np.ascontiguousarray(xte[b].T).astype(BF16),
                "wk": wk_dev, "wq": wq_dev, "wv": wv_dev,
                "cck": cck, "ssk": ssk, "ccq": ccq, "ssq": ssq,
            }
        )
    return in_maps


def kernel(x_image, x_text_emb, x_latex_mask, freqs_latex, freqs_img_x, freqs_img_y,
           Wk, Wq, Wv):
    del x_latex_mask  # unused in the reference
    from concourse.bass_utils import run_bass_kernel_spmd

    nc = build_program()
    in_maps = make_in_maps(
        x_image, x_text_emb, freqs_latex, freqs_img_x, freqs_img_y, Wk, Wq, Wv
    )
    res = run_bass_kernel_spmd(nc, in_maps, list(range(N_CORES)))
    return np.stack([res.results[b]["out"] for b in range(N_CORES)], axis=0)


# revision 22
# speedup vs baseline: 1.0148x; 1.0148x over previous
"""Trainium2 Bass kernel for nn_Cross_AttentionHead_withMask.

Cross-attention head: q = rope(x_text @ Wq.T), k = rope2d(x_image @ Wk.T),
v = x_image @ Wv.T, out = softmax(q k^T / sqrt(512)) v.
(x_latex_mask is accepted but unused — it is dead in the reference.)

Sharding: data-parallel over batch B=8, one batch per NeuronCore (8 cores).

Per-core device program (all matmuls bf16, accumulation/softmax stats fp32):
  - host ships x_image[b].T / x_text[b].T (bf16) so the contraction dim (C)
    lands on SBUF partitions without any on-device transposes
  - head dim is permuted to evens-then-odds so RoPE pairs become the row
    blocks [0:32] / [32:64]; rope = A*CC + partner(A)*SS (2 muls + 1 add)
  - scores computed transposed: weiT[t, s] = K2[:, t-tile].T @ Q2[:, s-chunk]
  - softmax exp is SPLIT between the Activation engine (exact exp) and the
    DVE (Schraudolph fast-exp: y = round(x*log2e*128 + (127*128 - C)) as
    int16, bit-viewed as bf16), alternating per step so neither engine
    paces the PE
  - attention-out: outT[h, s] += v_aug[t-tile].T @ expT, where v_aug carries
    a ones column so row 64 accumulates the softmax denominator for free;
    att-out lags the scores by TWO steps so exp latency is fully hidden
  - PSUM->SBUF projection copies and rope partner-swaps run on GpSimd (Pool),
    keeping Scalar exp-only and DVE mostly exp+rope-muls
  - epilogue: PE-transpose [65, 128] -> [128, 65], per-partition reciprocal
    of the Z column, tensor_scalar multiply, DMA out
"""
import numpy as np
from contextlib import ExitStack

import ml_dtypes

B, TQ, TK = 8, 2048, 4096
DIM_IMG, DIM_TXT, HS = 512, 128, 64
N_CORES = 8
SCALE = float(DIM_IMG) ** -0.5  # reference scales by sqrt(image embed dim)

# DVE fast-exp constants: exp(SCALE*x) ~= bitview_bf16(int16(round(
#   x * SCALE*log2(e)*128 + (127*128 - FEXP_C))))
FEXP_C = 6.0
FEXP_SCALE = SCALE * float(np.log2(np.e)) * 128.0
FEXP_BIAS = 127.0 * 128.0 - FEXP_C

BF16 = ml_dtypes.bfloat16

_prog_cache = {}


def _patch_tile_drain():
    """This walrus build rejects a Drain carrying >1 sem wait; split the
    TileContext exit waits onto one-wait NoOps."""
    import concourse.tile as tile
    from concourse import mybir
    from concourse.vector_clock import ScopedClock

    if getattr(tile.TileContext, "_drain_patched", False):
        return

    def _drain_and_barrier(self, tick_clock, wait_clock):
        nc = self.nc
        nop = nc.sync.nop()
        wait_clock.add_sem_waits(nop.ins, ScopedClock({None: tick_clock.global_clock}))
        si = nop.ins.sync_info
        waits = list(si.on_wait) if si is not None else []
        if len(waits) > 1:
            nop.ins.sync_info = mybir.SyncInfo(on_wait=[waits[0]], on_update=[])
            for w in waits[1:]:
                extra = nc.sync.nop()
                extra.ins.sync_info = mybir.SyncInfo(on_wait=[w], on_update=[])
        nc.sync.drain()
        nc.all_engine_barrier()
        assert self.sems is not None
        popped = nc._tile_sem_poison_stack.pop()
        assert popped is self._sem_poison
        nc.clear_and_free_semaphores(list(self.sems.allocated().values()))
        nc.all_engine_barrier()

    tile.TileContext._drain_and_barrier = _drain_and_barrier
    tile.TileContext._drain_patched = True


def _split_excess_waits(nc):
    """This walrus build caps sem waits per instruction (1 for DMA/Drain-style
    control instructions, 2 for compute). Move excess waits onto same-engine
    NoOps inserted right before the offending instruction — the engine queue
    is FIFO, so blocking dispatch on the NoOp is semantically equivalent."""
    from concourse import mybir

    ctr = 0
    for fn in nc.m.functions:
        for b in fn.blocks:
            il = b.instructions
            new = []
            changed = False
            for inst in il:
                si = inst.sync_info
                waits = list(si.on_wait) if si is not None else []
                lim = 1
                if len(waits) > lim:
                    for w in waits[lim:]:
                        nop = mybir.InstNoOp(name=f"wsplit-{ctr}", ins=[], outs=[])
                        ctr += 1
                        nop.engine = inst.engine
                        nop.sync_info = mybir.SyncInfo(on_wait=[w], on_update=[])
                        new.append(nop)
                    inst.sync_info = mybir.SyncInfo(
                        on_wait=waits[:lim], on_update=list(si.on_update)
                    )
                    changed = True
                new.append(inst)
            if changed:
                b.instructions = new


def build_program(split_waits=True):
    """Build the single-core Bass program (same program runs SPMD on 8 cores)."""
    key = ("nc", split_waits)
    if key in _prog_cache:
        return _prog_cache[key]

    _patch_tile_drain()
    import concourse.bass as bass
    import concourse.tile as tile
    from concourse import mybir
    from concourse.masks import make_identity

    FP = mybir.dt.float32
    BF = mybir.dt.bfloat16
    I16 = mybir.dt.int16
    MULT = mybir.AluOpType.mult
    ADD = mybir.AluOpType.add

    nc = bass.Bass("TRN2", target_bir_lowering=False, debug=False)
    xt = nc.dram_tensor("xt", [DIM_IMG, TK], BF, kind="ExternalInput").ap()
    xtt = nc.dram_tensor("xtt", [DIM_TXT, TQ], BF, kind="ExternalInput").ap()
    wk = nc.dram_tensor("wk", [DIM_IMG, HS], BF, kind="ExternalInput").ap()
    wq = nc.dram_tensor("wq", [DIM_TXT, HS], BF, kind="ExternalInput").ap()
    wv = nc.dram_tensor("wv", [DIM_IMG, HS], BF, kind="ExternalInput").ap()
    cck = nc.dram_tensor("cck", [HS, TK], BF, kind="ExternalInput").ap()
    ssk = nc.dram_tensor("ssk", [HS, TK], BF, kind="ExternalInput").ap()
    ccq = nc.dram_tensor("ccq", [HS, TQ], BF, kind="ExternalInput").ap()
    ssq = nc.dram_tensor("ssq", [HS, TQ], BF, kind="ExternalInput").ap()
    out = nc.dram_tensor("out", [TQ, HS], FP, kind="ExternalOutput").ap()

    Exp = mybir.ActivationFunctionType.Exp
    NC4 = DIM_IMG // 128  # 4 c-chunks
    NT = TK // 128  # 32 t-tiles
    NSC = TQ // 512  # 4 s-chunks

    with tile.TileContext(nc) as tc:
        with ExitStack() as ctx:
            const = ctx.enter_context(tc.tile_pool(name="const", bufs=1))
            pwp = ctx.enter_context(tc.tile_pool(name="pw", bufs=3, space="PSUM"))
            pop = ctx.enter_context(tc.tile_pool(name="po", bufs=2, space="PSUM"))
            esb = ctx.enter_context(tc.tile_pool(name="esb", bufs=5))
            osbp = ctx.enter_context(tc.tile_pool(name="osb", bufs=2))

            # ---- DMA rings: the 4 MB x_image.T alone on the fast HWDGE (sync)
            # ring; everything small on the gpsimd SWDGE ring ----
            xtt_sb = const.tile([128, TQ], BF, tag="xtt")
            nc.sync.dma_start(xtt_sb[:], xtt[:])
            xt_sb = [const.tile([128, TK], BF, tag=f"xt{ci}", name=f"xt_sb{ci}")
                     for ci in range(NC4)]
            # column-chunk order so k/v-proj can start as soon as the first
            # 512 columns of all four c-blocks have landed
            for j in range(8):
                cs = slice(j * 512, (j + 1) * 512)
                for ci in range(NC4):
                    nc.sync.dma_start(xt_sb[ci][:, cs], xt[ci * 128 : (ci + 1) * 128, cs])
            wq_sb = const.tile([128, HS], BF, tag="wq")
            nc.gpsimd.dma_start(wq_sb[:], wq[:])
            wk_sb = const.tile([128, NC4 * HS], BF, tag="wk")
            nc.gpsimd.dma_start(
                wk_sb[:].rearrange("p (a h) -> p a h", a=NC4),
                wk.rearrange("(a p) h -> p a h", p=128),
            )
            wv_sb = const.tile([128, NC4 * HS], BF, tag="wv")
            nc.gpsimd.dma_start(
                wv_sb[:].rearrange("p (a h) -> p a h", a=NC4),
                wv.rearrange("(a p) h -> p a h", p=128),
            )
            ccq_sb = const.tile([HS, TQ], BF, tag="ccq")
            nc.gpsimd.dma_start(ccq_sb[:], ccq[:])
            ssq_sb = const.tile([HS, TQ], BF, tag="ssq")
            nc.gpsimd.dma_start(ssq_sb[:], ssq[:])
            cck_sb = const.tile([HS, TK], BF, tag="cck")
            ssk_sb = const.tile([HS, TK], BF, tag="ssk")
            nc.gpsimd.dma_start(cck_sb[:], cck[:])
            nc.gpsimd.dma_start(ssk_sb[:], ssk[:])
            ident = const.tile([128, 128], FP, tag="ident")

            kt_pre = const.tile([HS, TK], BF, tag="ktpre")
            qt_pre = const.tile([HS, TQ], BF, tag="qtpre")
            v_half = [const.tile([128, NT * 65 // 2], BF, tag=f"vall{h}", name=f"vall{h}")
                      for h in range(2)]
            K2h = [const.tile([128, TK // 2], BF, tag=f"K2{h}", name=f"K2{h}")
                   for h in range(2)]
            pk = const.tile([HS, TK], BF, tag="pk")
            pq = const.tile([HS, TQ], BF, tag="pq")
            t1k = const.tile([HS, TK], BF, tag="t1k")
            t2k = const.tile([HS, TK], BF, tag="t2k")

            # ---- q projection + rope ----
            for j in range(TQ // 512):
                ps = pwp.tile([HS, 512], FP, tag="psw", name=f"psq{j}")
                nc.tensor.matmul(
                    ps[:], lhsT=wq_sb[:], rhs=xtt_sb[:, j * 512 : (j + 1) * 512],
                    start=True, stop=True,
                )
                nc.scalar.copy(qt_pre[:, j * 512 : (j + 1) * 512], ps[:])
            # rope partner-swaps and row-duplications are pure SBUF shuffles:
            # run them on the gpsimd DMA ring (free, and not stuck behind the
            # 4 MB x_image stream on the sync ring)
            nc.gpsimd.dma_start(pq[0:32, :], qt_pre[32:64, :])
            nc.gpsimd.dma_start(pq[32:64, :], qt_pre[0:32, :])
            t1q = const.tile([HS, TQ], BF, tag="t1q")
            nc.vector.tensor_mul(t1q[:], qt_pre[:], ccq_sb[:])
            t2q = const.tile([HS, TQ], BF, tag="t2q")
            nc.vector.tensor_mul(t2q[:], pq[:], ssq_sb[:])
            Q2 = const.tile([128, TQ], BF, tag="Q2")
            nc.vector.tensor_add(Q2[0:HS, :], t1q[:], t2q[:])
            nc.gpsimd.dma_start(Q2[HS:128, :], Q2[0:HS, :])

            def k_proj_chunk(j, cp):
                ps = pwp.tile([HS, 512], FP, tag="psw", name=f"psk{j}")
                for ci in range(NC4):
                    nc.tensor.matmul(
                        ps[:],
                        lhsT=wk_sb[:, ci * HS : (ci + 1) * HS],
                        rhs=xt_sb[ci][:, j * 512 : (j + 1) * 512],
                        start=(ci == 0), stop=(ci == NC4 - 1),
                    )
                cp(kt_pre[:, j * 512 : (j + 1) * 512], ps[:])

            def k_rope_half(h):
                cs = slice(h * (TK // 2), (h + 1) * (TK // 2))
                nc.gpsimd.dma_start(pk[0:32, cs], kt_pre[32:64, cs])
                nc.gpsimd.dma_start(pk[32:64, cs], kt_pre[0:32, cs])
                nc.vector.tensor_mul(t1k[:, cs], kt_pre[:, cs], cck_sb[:, cs])
                nc.vector.tensor_mul(t2k[:, cs], pk[:, cs], ssk_sb[:, cs])
                nc.vector.tensor_add(K2h[h][0:HS, :], t1k[:, cs], t2k[:, cs])
                nc.gpsimd.dma_start(K2h[h][HS:128, :], K2h[h][0:HS, :])

            def v_proj_tile(tt, cp):
                ps = pwp.tile([128, HS], FP, tag="psw", name=f"psv{tt}")
                for ci in range(NC4):
                    nc.tensor.matmul(
                        ps[:],
                        lhsT=xt_sb[ci][:, tt * 128 : (tt + 1) * 128],
                        rhs=wv_sb[:, ci * HS : (ci + 1) * HS],
                        start=(ci == 0), stop=(ci == NC4 - 1),
                    )
                vh, vo = v_half[tt // (NT // 2)], (tt % (NT // 2)) * 65
                cp(vh[:, vo : vo + HS], ps[:])

            # ---- attention machinery (flat pipeline over (sc, group) steps) ----
            GROUPS = [2] * 16
            psos = {}
            # att-out lags scores/exp by TWO steps so exp latency never
            # stalls the PE; pend queue holds up to 2 outstanding groups
            state = {"pend": [], "pend_epi": None}

            def att_group(pend):
                psc, pet, pgn, ptt = pend
                for j in range(pgn):
                    tj = ptt + j
                    vh, vo = v_half[tj // (NT // 2)], (tj % (NT // 2)) * 65
                    nc.tensor.matmul(
                        psos[psc][:],
                        lhsT=vh[:, vo : vo + 65],
                        rhs=pet[:, j * 512 : (j + 1) * 512],
                        start=(tj == 0), stop=(tj == NT - 1),
                    )

            def epilogue(psc):
                pso = psos.pop(psc)
                osb = osbp.tile([65, 512], FP, tag="osb", name=f"osb{psc}")
                nc.scalar.copy(osb[:], pso[:])
                out_sb = osbp.tile([128, 4 * HS], FP, tag="outsb", name=f"outsb{psc}")
                for j in range(4):
                    pst = pwp.tile([128, 65], FP, tag="psw", name=f"pst{psc}_{j}")
                    nc.tensor.transpose(
                        pst[:], osb[:, j * 128 : (j + 1) * 128], ident[0:65, 0:65]
                    )
                    zr = osbp.tile([128, 1], FP, tag="zr", name=f"zr{psc}_{j}")
                    nc.vector.reciprocal(zr[:], pst[:, HS : HS + 1])
                    nc.vector.tensor_scalar_mul(
                        out_sb[:, j * HS : (j + 1) * HS], pst[:, 0:HS], zr[:]
                    )
                nc.sync.dma_start(
                    out[psc * 512 : (psc + 1) * 512, :].rearrange(
                        "(j p) h -> p j h", p=128
                    ),
                    out_sb[:].rearrange("p (j h) -> p j h", j=4),
                )

            def flush_one():
                if not state["pend"]:
                    return
                pend = state["pend"].pop(0)
                psc, _, pgn, ptt = pend
                if psc not in psos:
                    psos[psc] = pop.tile([65, 512], FP, tag="pso", name=f"pso{psc}")
                att_group(pend)
                if state["pend_epi"] is not None:
                    epilogue(state["pend_epi"])
                    state["pend_epi"] = None
                if ptt + pgn == NT:
                    state["pend_epi"] = psc

            def att_steps(steps, exp_eng, extra=None, fillers=False):
                for si, (sc, gi) in enumerate(steps):
                    gn = GROUPS[gi]
                    tt = sum(GROUPS[:gi])
                    psw = pwp.tile([128, 1024], FP, tag="psw", name=f"psw{sc}_{gi}")
                    et = esb.tile([128, 1024], BF, tag="et", name=f"et{sc}_{gi}")
                    if fillers and si % 2 == 0:
                        # light HAM keep-warm filler on the psw slot-wait
                        nc.tensor.matmul(
                            psw[0:HS, 0:256], lhsT=wq_sb[:], rhs=xtt_sb[:, 0:256],
                            start=True, stop=True,
                        )
                    for j in range(gn):
                        tj = tt + j
                        kh = K2h[tj // (NT // 2)]
                        ko = (tj % (NT // 2)) * 128
                        rb = (j % 2) * HS  # alternate PE row groups: hides ldweights
                        nc.tensor.matmul(
                            psw[:, j * 512 : (j + 1) * 512],
                            lhsT=kh[rb : rb + HS, ko : ko + 128],
                            rhs=Q2[rb : rb + HS, sc * 512 : (sc + 1) * 512],
                            start=True, stop=True,
                        )
                    if exp_eng(si) == "scalar":
                        nc.scalar.activation(
                            et[:, : gn * 512], psw[:, : gn * 512], Exp, scale=SCALE
                        )
                    else:
                        # DVE fast-exp: int16(round(x*a + b)) bit-viewed as bf16
                        nc.vector.tensor_scalar(
                            et[:, : gn * 512].bitcast(I16),
                            psw[:, : gn * 512],
                            FEXP_SCALE, FEXP_BIAS, MULT, ADD,
                        )
                    if extra is not None:
                        extra(si)
                    if len(state["pend"]) >= 2:
                        flush_one()
                    state["pend"].append((sc, et, gn, tt))

            # ---- interleaved emission: first halves of k/v + rope, then the
            # first half of sc0's attention (exp engines start early),
            # then the second halves, then the rest of the attention ----
            # PE warm-up: dependency-free fillers right after q-proj so the
            # clock gate is already at 8/8 when x_image lands and k/v-proj run
            # Pre-phase: consume x_image chunk-by-chunk as the DMA delivers it
            # (~2.7us per 512-col chunk at the ~190GB/s HBM share). Interleave
            # dependency-free filler matmuls so the PE never looks idle to the
            # HAM clock governor during the DMA window — a throttled window
            # would also halve the DMA rate (death spiral).
            dummy = const.tile([128, 512], BF, tag="dummy")
            nc.vector.memset(dummy[:], 0.0)  # DVE is idle at kernel start
            garb0 = pwp.tile([HS, 512], FP, tag="psw", name="garb0")

            def garb(n):
                for fi in range(n):
                    nc.tensor.matmul(
                        garb0[:], lhsT=dummy[:, 0:HS], rhs=dummy[:],
                        start=True, stop=True,
                    )

            garb(14)
            for j in range(4):
                k_proj_chunk(j, nc.scalar.copy)
                for tt in range(4 * j, 4 * j + 4):
                    v_proj_tile(tt, nc.vector.tensor_copy)
                garb(4)
            k_rope_half(0)
            # identity + v_aug ones columns: needed from the first epilogue /
            # att-out; emitted here so they don't block the gpsimd DMA ring
            make_identity(nc, ident[:])
            nc.gpsimd.memset(v_half[0][:, HS :: 65], 1.0)
            nc.gpsimd.memset(v_half[1][:, HS :: 65], 1.0)

            # Phase A covers the half-0 t-tiles (gi 0-7) of sc0+sc1 — these
            # only need K2h[0] / v tiles 0-15, so they are data-free while the
            # x_image second half still streams in; the half-1 projections are
            # spread through A aligned with chunk arrival.
            def exp_eng_A(si):
                if si < 6:
                    return "scalar"
                return "vector" if si % 2 == 0 else "scalar"

            def exp_eng_rest(si):
                return "vector" if si % 2 == 0 else "scalar"

            def projA_extra(si):
                cp = (nc.vector.tensor_copy if exp_eng_A(si) == "scalar"
                      else nc.scalar.copy)
                if si in (2, 5, 8, 11):
                    k_proj_chunk(4 + (si - 2) // 3, cp)
                elif si in (3, 4, 6, 7, 9, 10, 13, 14):
                    idx = [3, 4, 6, 7, 9, 10, 13, 14].index(si)
                    v_proj_tile(16 + 2 * idx, cp)
                    v_proj_tile(17 + 2 * idx, cp)
                elif si == 12:
                    k_rope_half(1)

            att_steps([(sc, gi) for sc in range(2) for gi in range(8)],
                      exp_eng_A, extra=projA_extra, fillers=True)
            att_steps([(sc, gi) for sc in range(2) for gi in range(8, 16)],
                      exp_eng_rest)
            att_steps([(sc, gi) for sc in range(2, NSC) for gi in range(16)],
                      exp_eng_rest)
            # flush remaining groups + epilogues
            while state["pend"]:
                flush_one()
            if state["pend_epi"] is not None:
                epilogue(state["pend_epi"])

    if split_waits:
        _split_excess_waits(nc)
    _prog_cache[key] = nc
    return nc


def make_in_maps(x_image, x_text_emb, freqs_latex, freqs_img_x, freqs_img_y, Wk, Wq, Wv):
    """Host-side prep: transpose/cast activations, permute+transpose weights,
    build rope cos/sin tables in the permuted row layout."""
    perm = np.concatenate([np.arange(0, HS, 2), np.arange(1, HS, 2)])

    wk_dev = np.ascontiguousarray(np.asarray(Wk)[perm].T).astype(BF16)
    wq_dev = np.ascontiguousarray(np.asarray(Wq)[perm].T).astype(BF16)
    wv_dev = np.ascontiguousarray(np.asarray(Wv).T).astype(BF16)

    fx = np.asarray(freqs_img_x, dtype=np.float32)
    fy = np.asarray(freqs_img_y, dtype=np.float32)
    fl = np.asarray(freqs_latex, dtype=np.float32)
    ck_half = np.concatenate([fx[:, :, 0].T, fy[:, :, 0].T], axis=0)  # [32, TK]
    sk_half = np.concatenate([fx[:, :, 1].T, fy[:, :, 1].T], axis=0)
    cck = np.ascontiguousarray(np.concatenate([ck_half, ck_half], 0)).astype(BF16)
    ssk = np.ascontiguousarray(np.concatenate([-sk_half, sk_half], 0)).astype(BF16)
    cq_half = fl[:, :, 0].T  # [32, TQ]
    sq_half = fl[:, :, 1].T
    ccq = np.ascontiguousarray(np.concatenate([cq_half, cq_half], 0)).astype(BF16)
    ssq = np.ascontiguousarray(np.concatenate([-sq_half, sq_half], 0)).astype(BF16)

    xi = np.asarray(x_image, dtype=np.float32)
    xte = np.asarray(x_text_emb, dtype=np.float32)
    in_maps = []
    for b in range(N_CORES):
        in_maps.append(
            {
                "xt": np.ascontiguousarray(xi[b].T).astype(BF16),
                "xtt": np.ascontiguousarray(xte[b].T).astype(BF16),
                "wk": wk_dev, "wq": wq_dev, "wv": wv_dev,
                "cck": cck, "ssk": ssk, "ccq": ccq, "ssq": ssq,
            }
        )
    return in_maps


def kernel(x_image, x_text_emb, x_latex_mask, freqs_latex, freqs_img_x, freqs_img_y,
           Wk, Wq, Wv):
    del x_latex_mask  # unused in the reference
    from concourse.bass_utils import run_bass_kernel_spmd

    nc = build_program()
    in_maps = make_in_maps(
        x_image, x_text_emb, freqs_latex, freqs_img_x, freqs_img_y, Wk, Wq, Wv
    )
    res = run_bass_kernel_spmd(nc, in_maps, list(range(N_CORES)))
    return np.stack([res.results[b]["out"] for b in range(N_CORES)], axis=0)


# revision 26
# speedup vs baseline: 1.0241x; 1.0091x over previous
"""Trainium2 Bass kernel for nn_Cross_AttentionHead_withMask.

Cross-attention head: q = rope(x_text @ Wq.T), k = rope2d(x_image @ Wk.T),
v = x_image @ Wv.T, out = softmax(q k^T / sqrt(512)) v.
(x_latex_mask is accepted but unused — it is dead in the reference.)

Sharding: data-parallel over batch B=8, one batch per NeuronCore (8 cores).

Per-core device program (all matmuls bf16, accumulation/softmax stats fp32):
  - host ships x_image[b].T / x_text[b].T (bf16) so the contraction dim (C)
    lands on SBUF partitions without any on-device transposes
  - head dim is permuted to evens-then-odds so RoPE pairs become the row
    blocks [0:32] / [32:64]; rope = A*CC + partner(A)*SS (2 muls + 1 add)
  - scores computed transposed: weiT[t, s] = K2[:, t-tile].T @ Q2[:, s-chunk]
  - softmax exp is SPLIT between the Activation engine (exact exp) and the
    DVE (Schraudolph fast-exp: y = round(x*log2e*128 + (127*128 - C)) as
    int16, bit-viewed as bf16), alternating per step so neither engine
    paces the PE
  - attention-out: outT[h, s] += v_aug[t-tile].T @ expT, where v_aug carries
    a ones column so row 64 accumulates the softmax denominator for free;
    att-out lags the scores by TWO steps so exp latency is fully hidden
  - PSUM->SBUF projection copies and rope partner-swaps run on GpSimd (Pool),
    keeping Scalar exp-only and DVE mostly exp+rope-muls
  - epilogue: PE-transpose [65, 128] -> [128, 65], per-partition reciprocal
    of the Z column, tensor_scalar multiply, DMA out
"""
import numpy as np
from contextlib import ExitStack

import ml_dtypes

B, TQ, TK = 8, 2048, 4096
DIM_IMG, DIM_TXT, HS = 512, 128, 64
N_CORES = 8
SCALE = float(DIM_IMG) ** -0.5  # reference scales by sqrt(image embed dim)

# DVE fast-exp constants: exp(SCALE*x) ~= bitview_bf16(int16(round(
#   x * SCALE*log2(e)*128 + (127*128 - FEXP_C))))
FEXP_C = 6.0
FEXP_SCALE = SCALE * float(np.log2(np.e)) * 128.0
FEXP_BIAS = 127.0 * 128.0 - FEXP_C

BF16 = ml_dtypes.bfloat16

_prog_cache = {}


def _patch_tile_drain():
    """This walrus build rejects a Drain carrying >1 sem wait; split the
    TileContext exit waits onto one-wait NoOps."""
    import concourse.tile as tile
    from concourse import mybir
    from concourse.vector_clock import ScopedClock

    if getattr(tile.TileContext, "_drain_patched", False):
        return

    def _drain_and_barrier(self, tick_clock, wait_clock):
        nc = self.nc
        nop = nc.sync.nop()
        wait_clock.add_sem_waits(nop.ins, ScopedClock({None: tick_clock.global_clock}))
        si = nop.ins.sync_info
        waits = list(si.on_wait) if si is not None else []
        if len(waits) > 1:
            nop.ins.sync_info = mybir.SyncInfo(on_wait=[waits[0]], on_update=[])
            for w in waits[1:]:
                extra = nc.sync.nop()
                extra.ins.sync_info = mybir.SyncInfo(on_wait=[w], on_update=[])
        nc.sync.drain()
        nc.all_engine_barrier()
        assert self.sems is not None
        popped = nc._tile_sem_poison_stack.pop()
        assert popped is self._sem_poison
        nc.clear_and_free_semaphores(list(self.sems.allocated().values()))
        nc.all_engine_barrier()

    tile.TileContext._drain_and_barrier = _drain_and_barrier
    tile.TileContext._drain_patched = True


def _split_excess_waits(nc):
    """This walrus build caps sem waits per instruction (1 for DMA/Drain-style
    control instructions, 2 for compute). Move excess waits onto same-engine
    NoOps inserted right before the offending instruction — the engine queue
    is FIFO, so blocking dispatch on the NoOp is semantically equivalent."""
    from concourse import mybir

    ctr = 0
    for fn in nc.m.functions:
        for b in fn.blocks:
            il = b.instructions
            new = []
            changed = False
            for inst in il:
                si = inst.sync_info
                waits = list(si.on_wait) if si is not None else []
                lim = 1
                if len(waits) > lim:
                    for w in waits[lim:]:
                        nop = mybir.InstNoOp(name=f"wsplit-{ctr}", ins=[], outs=[])
                        ctr += 1
                        nop.engine = inst.engine
                        nop.sync_info = mybir.SyncInfo(on_wait=[w], on_update=[])
                        new.append(nop)
                    inst.sync_info = mybir.SyncInfo(
                        on_wait=waits[:lim], on_update=list(si.on_update)
                    )
                    changed = True
                new.append(inst)
            if changed:
                b.instructions = new


def build_program(split_waits=True):
    """Build the single-core Bass program (same program runs SPMD on 8 cores)."""
    key = ("nc", split_waits)
    if key in _prog_cache:
        return _prog_cache[key]

    _patch_tile_drain()
    import concourse.bass as bass
    import concourse.tile as tile
    from concourse import mybir
    from concourse.masks import make_identity

    FP = mybir.dt.float32
    BF = mybir.dt.bfloat16
    I16 = mybir.dt.int16
    MULT = mybir.AluOpType.mult
    ADD = mybir.AluOpType.add

    nc = bass.Bass("TRN2", target_bir_lowering=False, debug=False)
    xt = nc.dram_tensor("xt", [DIM_IMG, TK], BF, kind="ExternalInput").ap()
    xtt = nc.dram_tensor("xtt", [DIM_TXT, TQ], BF, kind="ExternalInput").ap()
    wk = nc.dram_tensor("wk", [DIM_IMG, HS], BF, kind="ExternalInput").ap()
    wq = nc.dram_tensor("wq", [DIM_TXT, HS], BF, kind="ExternalInput").ap()
    wv = nc.dram_tensor("wv", [DIM_IMG, HS], BF, kind="ExternalInput").ap()
    cck = nc.dram_tensor("cck", [HS, TK], BF, kind="ExternalInput").ap()
    ssk = nc.dram_tensor("ssk", [HS, TK], BF, kind="ExternalInput").ap()
    ccq = nc.dram_tensor("ccq", [HS, TQ], BF, kind="ExternalInput").ap()
    ssq = nc.dram_tensor("ssq", [HS, TQ], BF, kind="ExternalInput").ap()
    out = nc.dram_tensor("out", [TQ, HS], FP, kind="ExternalOutput").ap()

    Exp = mybir.ActivationFunctionType.Exp
    NC4 = DIM_IMG // 128  # 4 c-chunks
    NT = TK // 128  # 32 t-tiles
    NSC = TQ // 512  # 4 s-chunks

    with tile.TileContext(nc) as tc:
        with ExitStack() as ctx:
            const = ctx.enter_context(tc.tile_pool(name="const", bufs=1))
            pwp = ctx.enter_context(tc.tile_pool(name="pw", bufs=3, space="PSUM"))
            pop = ctx.enter_context(tc.tile_pool(name="po", bufs=2, space="PSUM"))
            esb = ctx.enter_context(tc.tile_pool(name="esb", bufs=5))
            osbp = ctx.enter_context(tc.tile_pool(name="osb", bufs=2))

            # ---- DMA rings: the 4 MB x_image.T alone on the fast HWDGE (sync)
            # ring; everything small on the gpsimd SWDGE ring ----
            xtt_sb = const.tile([128, TQ], BF, tag="xtt")
            nc.sync.dma_start(xtt_sb[:], xtt[:])
            xt_sb = [const.tile([128, TK], BF, tag=f"xt{ci}", name=f"xt_sb{ci}")
                     for ci in range(NC4)]
            # column-chunk order so k/v-proj can start as soon as the first
            # 512 columns of all four c-blocks have landed
            for j in range(8):
                cs = slice(j * 512, (j + 1) * 512)
                for ci in range(NC4):
                    nc.sync.dma_start(xt_sb[ci][:, cs], xt[ci * 128 : (ci + 1) * 128, cs])
            wq_sb = const.tile([128, HS], BF, tag="wq")
            nc.gpsimd.dma_start(wq_sb[:], wq[:])
            wk_sb = const.tile([128, NC4 * HS], BF, tag="wk")
            nc.gpsimd.dma_start(
                wk_sb[:].rearrange("p (a h) -> p a h", a=NC4),
                wk.rearrange("(a p) h -> p a h", p=128),
            )
            wv_sb = const.tile([128, NC4 * HS], BF, tag="wv")
            nc.gpsimd.dma_start(
                wv_sb[:].rearrange("p (a h) -> p a h", a=NC4),
                wv.rearrange("(a p) h -> p a h", p=128),
            )
            ccq_sb = const.tile([HS, TQ], BF, tag="ccq")
            nc.gpsimd.dma_start(ccq_sb[:], ccq[:])
            ssq_sb = const.tile([HS, TQ], BF, tag="ssq")
            nc.gpsimd.dma_start(ssq_sb[:], ssq[:])
            cck_sb = const.tile([HS, TK], BF, tag="cck")
            ssk_sb = const.tile([HS, TK], BF, tag="ssk")
            nc.gpsimd.dma_start(cck_sb[:], cck[:])
            nc.gpsimd.dma_start(ssk_sb[:], ssk[:])
            ident = const.tile([128, 128], FP, tag="ident")

            kt_pre = const.tile([HS, TK], BF, tag="ktpre")
            qt_pre = const.tile([HS, TQ], BF, tag="qtpre")
            v_half = [const.tile([128, NT * 65 // 2], BF, tag=f"vall{h}", name=f"vall{h}")
                      for h in range(2)]
            K2h = [const.tile([128, TK // 2], BF, tag=f"K2{h}", name=f"K2{h}")
                   for h in range(2)]
            pk = const.tile([HS, TK], BF, tag="pk")
            pq = const.tile([HS, TQ], BF, tag="pq")
            t1k = const.tile([HS, TK], BF, tag="t1k")
            t2k = const.tile([HS, TK], BF, tag="t2k")

            # ---- q projection + rope ----
            for j in range(TQ // 512):
                ps = pwp.tile([HS, 512], FP, tag="psw", name=f"psq{j}")
                nc.tensor.matmul(
                    ps[:], lhsT=wq_sb[:], rhs=xtt_sb[:, j * 512 : (j + 1) * 512],
                    start=True, stop=True,
                )
                nc.scalar.copy(qt_pre[:, j * 512 : (j + 1) * 512], ps[:])
            # rope partner-swaps and row-duplications are pure SBUF shuffles:
            # run them on the gpsimd DMA ring (free, and not stuck behind the
            # 4 MB x_image stream on the sync ring)
            nc.gpsimd.dma_start(pq[0:32, :], qt_pre[32:64, :])
            nc.gpsimd.dma_start(pq[32:64, :], qt_pre[0:32, :])
            t1q = const.tile([HS, TQ], BF, tag="t1q")
            nc.vector.tensor_mul(t1q[:], qt_pre[:], ccq_sb[:])
            t2q = const.tile([HS, TQ], BF, tag="t2q")
            nc.vector.tensor_mul(t2q[:], pq[:], ssq_sb[:])
            Q2 = const.tile([128, TQ], BF, tag="Q2")
            nc.vector.tensor_add(Q2[0:HS, :], t1q[:], t2q[:])
            nc.gpsimd.dma_start(Q2[HS:128, :], Q2[0:HS, :])

            def k_proj_chunk(j, cp):
                ps = pwp.tile([HS, 512], FP, tag="psw", name=f"psk{j}")
                for ci in range(NC4):
                    nc.tensor.matmul(
                        ps[:],
                        lhsT=wk_sb[:, ci * HS : (ci + 1) * HS],
                        rhs=xt_sb[ci][:, j * 512 : (j + 1) * 512],
                        start=(ci == 0), stop=(ci == NC4 - 1),
                    )
                cp(kt_pre[:, j * 512 : (j + 1) * 512], ps[:])

            def k_rope_half(h):
                cs = slice(h * (TK // 2), (h + 1) * (TK // 2))
                nc.gpsimd.dma_start(pk[0:32, cs], kt_pre[32:64, cs])
                nc.gpsimd.dma_start(pk[32:64, cs], kt_pre[0:32, cs])
                nc.vector.tensor_mul(t1k[:, cs], kt_pre[:, cs], cck_sb[:, cs])
                nc.vector.tensor_mul(t2k[:, cs], pk[:, cs], ssk_sb[:, cs])
                nc.vector.tensor_add(K2h[h][0:HS, :], t1k[:, cs], t2k[:, cs])
                nc.gpsimd.dma_start(K2h[h][HS:128, :], K2h[h][0:HS, :])

            def v_proj_tile(tt, cp):
                ps = pwp.tile([128, HS], FP, tag="psw", name=f"psv{tt}")
                for ci in range(NC4):
                    nc.tensor.matmul(
                        ps[:],
                        lhsT=xt_sb[ci][:, tt * 128 : (tt + 1) * 128],
                        rhs=wv_sb[:, ci * HS : (ci + 1) * HS],
                        start=(ci == 0), stop=(ci == NC4 - 1),
                    )
                vh, vo = v_half[tt // (NT // 2)], (tt % (NT // 2)) * 65
                cp(vh[:, vo : vo + HS], ps[:])

            # ---- attention machinery (flat pipeline over (sc, group) steps) ----
            GROUPS = [2] * 16
            psos = {}
            # att-out lags scores/exp by TWO steps so exp latency never
            # stalls the PE; pend queue holds up to 2 outstanding groups
            state = {"pend": [], "epi_q": []}

            def att_group(pend):
                psc, pet, pgn, ptt = pend
                for j in range(pgn):
                    tj = ptt + j
                    vh, vo = v_half[tj // (NT // 2)], (tj % (NT // 2)) * 65
                    nc.tensor.matmul(
                        psos[psc][:],
                        lhsT=vh[:, vo : vo + 65],
                        rhs=pet[:, j * 512 : (j + 1) * 512],
                        start=(tj == 0), stop=(tj == NT - 1),
                    )

            def epilogue_piece(ctx2):
                # one 128-row slice of an s-chunk's epilogue: smooths the
                # engine queues vs a 2.5us burst (which starves the exp
                # engines -> psw slot stall -> HAM throttle)
                psc, j, pso, osb = ctx2
                if j == 0:
                    nc.scalar.copy(osb[:], pso[:])
                pst = pwp.tile([128, 65], FP, tag="psw", name=f"pst{psc}_{j}")
                nc.tensor.transpose(
                    pst[:], osb[:, j * 128 : (j + 1) * 128], ident[0:65, 0:65]
                )
                pst_sb = osbp.tile([128, 65], FP, tag="pstsb", name=f"pstsb{psc}_{j}")
                nc.scalar.copy(pst_sb[:], pst[:])
                zr = osbp.tile([128, 1], FP, tag="zr", name=f"zr{psc}_{j}")
                nc.vector.reciprocal(zr[:], pst_sb[:, HS : HS + 1])
                out_sb = osbp.tile([128, HS], FP, tag="outsb", name=f"outsb{psc}_{j}")
                nc.gpsimd.tensor_scalar_mul(out_sb[:], pst_sb[:, 0:HS], zr[:])
                nc.sync.dma_start(
                    out[psc * 512 + j * 128 : psc * 512 + (j + 1) * 128, :],
                    out_sb[:],
                )

            def flush_one():
                if not state["pend"]:
                    return
                pend = state["pend"].pop(0)
                psc, _, pgn, ptt = pend
                if psc not in psos:
                    psos[psc] = pop.tile([65, 512], FP, tag="pso", name=f"pso{psc}")
                att_group(pend)
                if state["epi_q"]:
                    epilogue_piece(state["epi_q"].pop(0))
                if ptt + pgn == NT:
                    pso = psos.pop(psc)
                    osb = osbp.tile([65, 512], FP, tag="osb", name=f"osb{psc}")
                    state["epi_q"].extend((psc, j, pso, osb) for j in range(4))

            def att_steps(steps, exp_eng, extra=None, fillers=False):
                for si, (sc, gi) in enumerate(steps):
                    gn = GROUPS[gi]
                    tt = sum(GROUPS[:gi])
                    psw = pwp.tile([128, 1024], FP, tag="psw", name=f"psw{sc}_{gi}")
                    et = esb.tile([128, 1024], BF, tag="et", name=f"et{sc}_{gi}")
                    if fillers and si % 2 == 0:
                        # light HAM keep-warm filler on the psw slot-wait
                        nc.tensor.matmul(
                            psw[0:HS, 0:256], lhsT=wq_sb[:], rhs=xtt_sb[:, 0:256],
                            start=True, stop=True,
                        )
                    for j in range(gn):
                        tj = tt + j
                        kh = K2h[tj // (NT // 2)]
                        ko = (tj % (NT // 2)) * 128
                        rb = (j % 2) * HS  # alternate PE row groups: hides ldweights
                        nc.tensor.matmul(
                            psw[:, j * 512 : (j + 1) * 512],
                            lhsT=kh[rb : rb + HS, ko : ko + 128],
                            rhs=Q2[rb : rb + HS, sc * 512 : (sc + 1) * 512],
                            start=True, stop=True,
                        )
                    if exp_eng(si) == "scalar":
                        nc.scalar.activation(
                            et[:, : gn * 512], psw[:, : gn * 512], Exp, scale=SCALE
                        )
                    else:
                        # DVE fast-exp: int16(round(x*a + b)) bit-viewed as bf16
                        nc.vector.tensor_scalar(
                            et[:, : gn * 512].bitcast(I16),
                            psw[:, : gn * 512],
                            FEXP_SCALE, FEXP_BIAS, MULT, ADD,
                        )
                    if extra is not None:
                        extra(si)
                    if len(state["pend"]) >= 2:
                        flush_one()
                    state["pend"].append((sc, et, gn, tt))

            # ---- interleaved emission: first halves of k/v + rope, then the
            # first half of sc0's attention (exp engines start early),
            # then the second halves, then the rest of the attention ----
            # PE warm-up: dependency-free fillers right after q-proj so the
            # clock gate is already at 8/8 when x_image lands and k/v-proj run
            # Pre-phase: consume x_image chunk-by-chunk as the DMA delivers it
            # (~2.7us per 512-col chunk at the ~190GB/s HBM share). Interleave
            # dependency-free filler matmuls so the PE never looks idle to the
            # HAM clock governor during the DMA window — a throttled window
            # would also halve the DMA rate (death spiral).
            dummy = const.tile([128, 512], BF, tag="dummy")
            nc.vector.memset(dummy[:], 0.0)  # DVE is idle at kernel start
            garb0 = pwp.tile([HS, 512], FP, tag="psw", name="garb0")

            def garb(n):
                for fi in range(n):
                    nc.tensor.matmul(
                        garb0[:], lhsT=dummy[:, 0:HS], rhs=dummy[:],
                        start=True, stop=True,
                    )

            garb(14)
            for j in range(4):
                k_proj_chunk(j, nc.scalar.copy)
                for tt in range(4 * j, 4 * j + 4):
                    v_proj_tile(tt, nc.vector.tensor_copy)
                garb(4)
            k_rope_half(0)
            # identity + v_aug ones columns: needed from the first epilogue /
            # att-out; emitted here so they don't block the gpsimd DMA ring
            make_identity(nc, ident[:])
            nc.gpsimd.memset(v_half[0][:, HS :: 65], 1.0)
            nc.gpsimd.memset(v_half[1][:, HS :: 65], 1.0)

            # Phase A covers the half-0 t-tiles (gi 0-7) of sc0+sc1 — these
            # only need K2h[0] / v tiles 0-15, so they are data-free while the
            # x_image second half still streams in; the half-1 projections are
            # spread through A aligned with chunk arrival.
            def exp_eng_A(si):
                if si < 6:
                    return "scalar"
                return "vector" if si % 2 == 0 else "scalar"

            def exp_eng_rest(si):
                return "vector" if si % 2 == 0 else "scalar"

            def projA_extra(si):
                cp = (nc.vector.tensor_copy if exp_eng_A(si) == "scalar"
                      else nc.scalar.copy)
                if si in (2, 4, 6, 8):
                    k_proj_chunk(4 + (si - 2) // 2, cp)
                elif si == 9:
                    k_rope_half(1)
                elif si in (3, 5, 7, 10, 11, 12, 13, 14):
                    idx = [3, 5, 7, 10, 11, 12, 13, 14].index(si)
                    v_proj_tile(16 + 2 * idx, cp)
                    v_proj_tile(17 + 2 * idx, cp)

            att_steps([(sc, gi) for sc in range(2) for gi in range(8)],
                      exp_eng_A, extra=projA_extra, fillers=True)
            att_steps([(sc, gi) for sc in range(2) for gi in range(8, 16)],
                      exp_eng_rest, fillers=True)
            att_steps([(sc, gi) for sc in range(2, NSC) for gi in range(16)],
                      exp_eng_rest)
            # flush remaining groups + epilogues
            while state["pend"]:
                flush_one()
            while state["epi_q"]:
                epilogue_piece(state["epi_q"].pop(0))

    if split_waits:
        _split_excess_waits(nc)
    _prog_cache[key] = nc
    return nc


def make_in_maps(x_image, x_text_emb, freqs_latex, freqs_img_x, freqs_img_y, Wk, Wq, Wv):
    """Host-side prep: transpose/cast activations, permute+transpose weights,
    build rope cos/sin tables in the permuted row layout."""
    perm = np.concatenate([np.arange(0, HS, 2), np.arange(1, HS, 2)])

    wk_dev = np.ascontiguousarray(np.asarray(Wk)[perm].T).astype(BF16)
    wq_dev = np.ascontiguousarray(np.asarray(Wq)[perm].T).astype(BF16)
    wv_dev = np.ascontiguousarray(np.asarray(Wv).T).astype(BF16)

    fx = np.asarray(freqs_img_x, dtype=np.float32)
    fy = np.asarray(freqs_img_y, dtype=np.float32)
    fl = np.asarray(freqs_latex, dtype=np.float32)
    ck_half = np.concatenate([fx[:, :, 0].T, fy[:, :, 0].T], axis=0)  # [32, TK]
    sk_half = np.concatenate([fx[:, :, 1].T, fy[:, :, 1].T], axis=0)
    cck = np.ascontiguousarray(np.concatenate([ck_half, ck_half], 0)).astype(BF16)
    ssk = np.ascontiguousarray(np.concatenate([-sk_half, sk_half], 0)).astype(BF16)
    cq_half = fl[:, :, 0].T  # [32, TQ]
    sq_half = fl[:, :, 1].T
    ccq = np.ascontiguousarray(np.concatenate([cq_half, cq_half], 0)).astype(BF16)
    ssq = np.ascontiguousarray(np.concatenate([-sq_half, sq_half], 0)).astype(BF16)

    xi = np.asarray(x_image, dtype=np.float32)
    xte = np.asarray(x_text_emb, dtype=np.float32)
    in_maps = []
    for b in range(N_CORES):
        in_maps.append(
            {
                "xt": np.ascontiguousarray(xi[b].T).astype(BF16),
                "xtt": np.ascontiguousarray(xte[b].T).astype(BF16),
                "wk": wk_dev, "wq": wq_dev, "wv": wv_dev,
                "cck": cck, "ssk": ssk, "ccq": ccq, "ssq": ssq,
            }
        )
    return in_maps


def kernel(x_image, x_text_emb, x_latex_mask, freqs_latex, freqs_img_x, freqs_img_y,
           Wk, Wq, Wv):
    del x_latex_mask  # unused in the reference
    from concourse.bass_utils import run_bass_kernel_spmd

    nc = build_program()
    in_maps = make_in_maps(
        x_image, x_text_emb, freqs_latex, freqs_img_x, freqs_img_y, Wk, Wq, Wv
    )
    res = run_bass_kernel_spmd(nc, in_maps, list(range(N_CORES)))
    return np.stack([res.results[b]["out"] for b in range(N_CORES)], axis=0)
